# revision 25
# baseline (speedup 1.0000x reference)
"""Causal self-attention (B=2, S=2048, E=2048, H=16) on 8 TRN2 NeuronCores.

Sharding: 2-way batch x 4-way head-group tensor parallel.
Core c handles batch c//4 and heads [4*(c%4), 4*(c%4)+4).

Single-pass bf16 design (the rel-err gate is 2e-2; bf16 lands ~4e-3):
  - QKV projection from host-pretransposed X^T (bf16), feature-major
    qT/kT/vT [128hd, S] bf16 kept in SBUF. SCALE folded into Wq/bq host-side.
  - Attention per head: k-major scores (stationary kT block, moving qT) ->
    transposed-causal mask -> exp to bf16 expPT (k-major = the PV moving
    layout: no P transposes). Row sums via a ones-column matmul; the PV
    output is normalized late, with the per-q reciprocal broadcast across
    partitions by a tiny K=1 matmul. rs/PV matmuls run one kb behind the
    scores (software pipeline) so the PE never waits on exp.
  - The TRN2 PE clock ramps (0.65 -> 1.2 -> 2.4 GHz after 3us continuous
    busy), so gaps are doubly expensive: attention of head h is interleaved
    with the QKV projection of head h+1 to keep the PE queue full.
    PSUM: qkv 4 banks + scores 1 + pv 2 (chunk-pair split) + rs/bc 1 = 8.
  - Out projection: stationary attO blocks (feature-major), moving W_out
    rows; fp32 partials to DRAM.

Host: per batch X^T bf16 (shared by 4 cores), per head-group W slices bf16;
sum the 4 head-group partials per batch and add (b_out + b_v @ W_out) once
(softmax rows sum to 1, so the v-bias term is a constant row vector).
"""

from contextlib import ExitStack

import ml_dtypes
import numpy as np

import concourse.bass as bass
import concourse.tile as tile
from concourse import bacc, bass_utils, mybir
from concourse.masks import make_identity

FP = mybir.dt.float32
BF = mybir.dt.bfloat16
AF = mybir.ActivationFunctionType

B = 2
S = 2048
E = 2048
H = 16
HD = 128
NCORES = 8
HG = 4  # head-group axis (tensor parallel)
H_LOC = H // HG  # 4 heads per core
FLOC = H_LOC * HD  # 512 local features per q/k/v
SCALE = 1.0 / float(np.sqrt(HD))
NEG = -1.0e30

PROFILE = False
LAST_EXEC_NS = None
LAST_RESULTS = None


def _emit(nc, S=S, E=E):
    NB = S // 128  # token blocks
    EB = E // 128  # embed blocks
    NC = S // 512  # 512-wide q chunks

    xT = nc.dram_tensor("xT", [E, S], BF, kind="ExternalInput").ap()
    wq = nc.dram_tensor("wq", [E, FLOC], BF, kind="ExternalInput").ap()
    wk = nc.dram_tensor("wk", [E, FLOC], BF, kind="ExternalInput").ap()
    wv = nc.dram_tensor("wv", [E, FLOC], BF, kind="ExternalInput").ap()
    bqs = nc.dram_tensor("bqs", [FLOC, 1], FP, kind="ExternalInput").ap()  # *SCALE
    bk = nc.dram_tensor("bk", [FLOC, 1], FP, kind="ExternalInput").ap()
    wo = nc.dram_tensor("wo", [FLOC, E], BF, kind="ExternalInput").ap()
    out = nc.dram_tensor("out", [S, E], FP, kind="ExternalOutput").ap()

    with tile.TileContext(nc) as tc, ExitStack() as top:
        cst = top.enter_context(tc.tile_pool(name="cst", bufs=1))
        ident_bf = cst.tile([128, 128], BF, name="identbf", tag="identbf")
        make_identity(nc, ident_bf[:])
        # transposed causal mask: keep (0) where q(free) >= k(part), NEG below
        maskT = cst.tile([128, 128], FP, name="maskT", tag="maskT")
        nc.gpsimd.memset(maskT[:], 0.0)
        nc.gpsimd.affine_select(
            out=maskT[:],
            in_=maskT[:],
            compare_op=mybir.AluOpType.is_ge,
            fill=NEG,
            base=0,
            pattern=[[1, 128]],  # +1 * free index
            channel_multiplier=-1,  # -1 * partition index
        )
        # 32 identical ones columns: rowsum matmuls write 32 equal psum rows,
        # so the copy/reciprocal after run 32-wide instead of single-partition
        ones_col = cst.tile([128, 32], BF, name="onescol", tag="onescol")
        nc.vector.memset(ones_col[:], 1.0)
        ones_row = cst.tile([1, 128], FP, name="onesrow", tag="onesrow")
        nc.vector.memset(ones_row[:], 1.0)
        bq_sb = cst.tile([128, H_LOC], FP, name="bq", tag="bq")
        bk_sb = cst.tile([128, H_LOC], FP, name="bk", tag="bk")
        for f in range(H_LOC):
            nc.sync.dma_start(bq_sb[:, f : f + 1], bqs[128 * f : 128 * (f + 1), :])
            nc.sync.dma_start(bk_sb[:, f : f + 1], bk[128 * f : 128 * (f + 1), :])

        # SBUF-resident tensors
        qkv_pool = top.enter_context(tc.tile_pool(name="qkvT", bufs=1))
        qT = [qkv_pool.tile([128, S], BF, name=f"qT{h}", tag=f"qT{h}") for h in range(H_LOC)]
        kT = [qkv_pool.tile([128, S], BF, name=f"kT{h}", tag=f"kT{h}") for h in range(H_LOC)]
        # vT[h] is dead once vsb(h) is built: rotate through 2 buffers
        vt_pool = top.enter_context(tc.tile_pool(name="vt", bufs=2))
        vts = {}
        att_pool = top.enter_context(tc.tile_pool(name="att", bufs=1))
        attO = [
            att_pool.tile([128, S], BF, name=f"attO{h}", tag=f"attO{h}")
            for h in range(H_LOC)
        ]
        xt_pool = top.enter_context(tc.tile_pool(name="xt", bufs=1))
        xts = [
            xt_pool.tile([128, S], BF, name=f"xt{e}", tag=f"xt{e}") for e in range(EB)
        ]
        wpool = top.enter_context(tc.tile_pool(name="w", bufs=1))
        wsb = [
            wpool.tile([128, EB * FLOC], BF, name=f"wsb{wi}", tag=f"wsb{wi}")
            for wi in range(3)
        ]
        wo_pool = top.enter_context(tc.tile_pool(name="wo", bufs=1))
        wos = [
            wo_pool.tile([128, E], BF, name=f"wo{h}", tag=f"wo{h}")
            for h in range(H_LOC)
        ]
        vsb_pool = top.enter_context(tc.tile_pool(name="vsb", bufs=1))
        ept_pool = top.enter_context(tc.tile_pool(name="ept", bufs=2))
        rsb_pool = top.enter_context(tc.tile_pool(name="rsb", bufs=1))
        bcs_pool = top.enter_context(tc.tile_pool(name="bcs", bufs=1))
        ostg = top.enter_context(tc.tile_pool(name="ostg", bufs=2))

        # PSUM: 4 + 1 + 2 + 1 = 8 banks
        ps_qkv = top.enter_context(tc.tile_pool(name="ps_qkv", bufs=4, space="PSUM"))
        ps_sc = top.enter_context(tc.tile_pool(name="ps_sc", bufs=1, space="PSUM"))
        ps_pv = top.enter_context(tc.tile_pool(name="ps_pv", bufs=2, space="PSUM"))
        ps_rs = top.enter_context(tc.tile_pool(name="ps_rs", bufs=1, space="PSUM"))

        # ---- DMA emission in first-use order ----
        for e in range(EB):
            nc.sync.dma_start(
                wsb[0][:, FLOC * e : FLOC * (e + 1)], wq[128 * e : 128 * (e + 1), :]
            )
            nc.sync.dma_start(xts[e][:], xT[128 * e : 128 * (e + 1), :])
        for wi, wsrc in ((1, wk), (2, wv)):
            for e in range(EB):
                nc.sync.dma_start(
                    wsb[wi][:, FLOC * e : FLOC * (e + 1)],
                    wsrc[128 * e : 128 * (e + 1), :],
                )
        for h in range(H_LOC):
            nc.sync.dma_start(wos[h][:], wo[128 * h : 128 * (h + 1), :])

        # ---- emission units ----
        def qkv_units(h):
            """QKV projection for head h: one unit per (which, e) plus drains."""
            units = []
            state = {}

            def alloc(which):
                def u():
                    state[which] = [
                        ps_qkv.tile([128, 512], FP, name="psq", tag="psq")
                        for _ in range(NC)
                    ]

                return u

            def mm(which, e):
                def u():
                    psums = state[which]
                    wt = wsb[which][:, FLOC * e + 128 * h : FLOC * e + 128 * (h + 1)]
                    for sc in range(NC):
                        nc.tensor.matmul(
                            psums[sc][:],
                            wt,
                            xts[e][:, 512 * sc : 512 * (sc + 1)],
                            start=(e == 0),
                            stop=(e == EB - 1),
                        )

                return u

            def drain(which):
                def u():
                    psums = state[which]
                    if which == 2:
                        vts[h] = vt_pool.tile([128, S], BF, name="vT", tag="vT")
                    dst = (qT[h], kT[h], vts.get(h))[which]
                    for sc in range(NC):
                        sl = slice(512 * sc, 512 * (sc + 1))
                        if which == 0:
                            nc.vector.tensor_scalar_add(
                                dst[:, sl], psums[sc][:], bq_sb[:, h : h + 1]
                            )
                        elif which == 1:
                            nc.vector.tensor_scalar_add(
                                dst[:, sl], psums[sc][:], bk_sb[:, h : h + 1]
                            )
                        else:
                            nc.scalar.activation(dst[:, sl], psums[sc][:], AF.Copy)

                return u

            for which in range(3):
                units.append(alloc(which))
                for e in range(EB):
                    units.append(mm(which, e))
                units.append(drain(which))
            return units

        def attn_units(h, sc_pool, sc_tag):
            """Attention for head h, chunk-pair split (pv uses 2 banks)."""
            units = []
            state = {}

            def vsb_tr(mg):
                def u():
                    if mg == 0:
                        state["vsb"] = vsb_pool.tile(
                            [128, S], BF, name="vsb", tag="vsb"
                        )
                    pv = sc_pool.tile([128, 512], BF, name="pst", tag=sc_tag)
                    for m in range(4):
                        i = 4 * mg + m
                        nc.tensor.transpose(
                            pv[:, 128 * m : 128 * (m + 1)],
                            vts[h][:, 128 * i : 128 * (i + 1)],
                            ident_bf[:],
                        )
                    nc.scalar.activation(
                        state["vsb"][:, 512 * mg : 512 * (mg + 1)], pv[:], AF.Copy
                    )

                return u

            for mg in range(NB // 4):
                units.append(vsb_tr(mg))

            def pair_begin(chunks):
                def u():
                    state["pv"] = {
                        c: ps_pv.tile([128, 512], FP, name="pspv", tag="pspv")
                        for c in chunks
                    }
                    state["rs"] = ps_rs.tile([128, 512], FP, name="psrs", tag="psrs")
                    state["ept"] = {}

                return u

            def scores(kb, c, chunks):
                def u():
                    k0 = 128 * kb
                    cmin = kb // 4
                    off = 512 * chunks[0]
                    if c == max(chunks[0], cmin):
                        state["ept"][kb] = ept_pool.tile(
                            [128, 1024], BF, name="ept", tag="ept"
                        )
                    ept = state["ept"][kb]
                    qlo = max(0, k0 - 512 * c)
                    q0 = 512 * c
                    scp = sc_pool.tile([128, 512], FP, name="pssc", tag=sc_tag)
                    nc.tensor.matmul(
                        scp[:, qlo:512],
                        kT[h][:, k0 : k0 + 128],
                        qT[h][:, q0 + qlo : q0 + 512],
                        start=True,
                        stop=True,
                    )
                    if c == cmin:
                        nc.vector.tensor_add(
                            scp[:, qlo : qlo + 128],
                            scp[:, qlo : qlo + 128],
                            maskT[:],
                        )
                    nc.scalar.activation(
                        ept[:, q0 + qlo - off : q0 + 512 - off],
                        scp[:, qlo:512],
                        AF.Exp,
                    )

                return u

            def rspv(kb, c, chunks):
                def u():
                    k0 = 128 * kb
                    off = 512 * chunks[0]
                    ept = state["ept"][kb]
                    rs_ps = state["rs"]
                    qlo = max(0, k0 - 512 * c)
                    q0 = 512 * c
                    rp = 32 * (c % 2)
                    nc.tensor.matmul(
                        rs_ps[rp : rp + 32, qlo:512],
                        ones_col[:],
                        ept[:, q0 + qlo - off : q0 + 512 - off],
                        start=(kb == 0),
                        stop=(kb == 4 * c + 3),
                    )
                    nc.tensor.matmul(
                        state["pv"][c][:, qlo:512],
                        state["vsb"][:, k0 : k0 + 128],
                        ept[:, q0 + qlo - off : q0 + 512 - off],
                        start=(kb == 0),
                        stop=(kb == 4 * c + 3),
                    )

                return u

            def normalize(chunks):
                def u():
                    rs_ps = state["rs"]
                    rs_sb = rsb_pool.tile([32, 1024], FP, name="rssb", tag="rssb")
                    for j in range(2):
                        nc.vector.reciprocal(
                            rs_sb[:, 512 * j : 512 * (j + 1)],
                            rs_ps[32 * j : 32 * (j + 1), :],
                        )
                    for j, c in enumerate(chunks):
                        bc_ps = ps_rs.tile([128, 512], FP, name="psbc", tag="psrs")
                        nc.tensor.matmul(
                            bc_ps[:],
                            ones_row[:],
                            rs_sb[0:1, 512 * j : 512 * (j + 1)],
                            start=True,
                            stop=True,
                        )
                        bc_sb = bcs_pool.tile([128, 512], FP, name="bcsb", tag="bcsb")
                        nc.scalar.activation(bc_sb[:], bc_ps[:], AF.Copy)
                        nc.vector.tensor_mul(
                            attO[h][:, 512 * c : 512 * (c + 1)],
                            state["pv"][c][:],
                            bc_sb[:],
                        )

                return u

            for chunks, nkb in (((0, 1), 8), ((2, 3), NB)):
                units.append(pair_begin(chunks))
                for kb in range(nkb + 1):
                    if kb < nkb:
                        cmin = kb // 4
                        for c in chunks:
                            if c >= cmin:
                                units.append(scores(kb, c, chunks))
                    if kb > 0:
                        cmin = (kb - 1) // 4
                        for c in chunks:
                            if c >= cmin:
                                units.append(rspv(kb - 1, c, chunks))
                units.append(normalize(chunks))
            return units

        # ---- emit: qkv(0), then attn(h) interleaved with qkv(h+1) ----
        for u in qkv_units(0):
            u()
        for h in range(H_LOC):
            if h < 3:
                sc_pool, sc_tag = ps_sc, "pssc"
                qs = qkv_units(h + 1)
            else:
                # qkv banks are free once head 3's projection is done
                sc_pool, sc_tag = ps_qkv, "psq"
                qs = []
            ats = attn_units(h, sc_pool, sc_tag)
            qi = 0
            for i, at in enumerate(ats):
                at()
                tgt = (i + 1) * len(qs) // len(ats)
                while qi < tgt:
                    qs[qi]()
                    qi += 1
            while qi < len(qs):
                qs[qi]()
                qi += 1

        # ---------------- output projection ----------------
        with nc.named_scope("outproj"):
            nec = E // 512
            for i in range(NB):
                if i % 2 == 0:
                    psums = [
                        ps_qkv.tile([128, 512], FP, name="pso", tag="psq")
                        for _ in range(nec)
                    ]
                else:
                    psums = [
                        ps_pv.tile([128, 512], FP, name="pso", tag="pspv"),
                        ps_pv.tile([128, 512], FP, name="pso", tag="pspv"),
                        ps_sc.tile([128, 512], FP, name="pso", tag="pssc"),
                        ps_rs.tile([128, 512], FP, name="pso", tag="psrs"),
                    ]
                for h in range(H_LOC):
                    ah_blk = attO[h][:, 128 * i : 128 * (i + 1)]
                    for c in range(nec):
                        nc.tensor.matmul(
                            psums[c][:],
                            ah_blk,
                            wos[h][:, 512 * c : 512 * (c + 1)],
                            start=(h == 0),
                            stop=(h == H_LOC - 1),
                        )
                for c in range(nec):
                    ot = ostg.tile([128, 512], FP, name="ostg", tag="ostg")
                    if c % 2 == 0:
                        nc.vector.tensor_copy(ot[:], psums[c][:])
                    else:
                        nc.scalar.activation(ot[:], psums[c][:], AF.Copy)
                    nc.sync.dma_start(
                        out[128 * i : 128 * (i + 1), 512 * c : 512 * (c + 1)],
                        ot[:],
                    )


_NC_CACHE = None


def _get_nc():
    global _NC_CACHE
    if _NC_CACHE is None:
        nc = bacc.Bacc(
            "TRN2",
            target_bir_lowering=False,
            debug=False,
            num_devices=1,
            enable_asserts=False,
        )
        _emit(nc)
        nc.compile()
        _NC_CACHE = nc
    return _NC_CACHE


def make_in_maps(inX, W_qkv, b_qkv, W_out):
    bf = ml_dtypes.bfloat16
    xTs = [np.ascontiguousarray(inX[b].T.astype(bf)) for b in range(B)]
    in_maps = []
    for c in range(NCORES):
        b = c // HG
        hg = c % HG
        sl = slice(FLOC * hg, FLOC * (hg + 1))
        in_maps.append(
            {
                "xT": xTs[b],
                "wq": np.ascontiguousarray(
                    (W_qkv[:, 0:E][:, sl] * SCALE).astype(bf)
                ),
                "wk": np.ascontiguousarray(W_qkv[:, E : 2 * E][:, sl].astype(bf)),
                "wv": np.ascontiguousarray(W_qkv[:, 2 * E : 3 * E][:, sl].astype(bf)),
                "bqs": np.ascontiguousarray(
                    (b_qkv[0:E][sl] * SCALE).reshape(FLOC, 1).astype(np.float32)
                ),
                "bk": np.ascontiguousarray(
                    b_qkv[E : 2 * E][sl].reshape(FLOC, 1).astype(np.float32)
                ),
                "wo": np.ascontiguousarray(W_out[sl, :].astype(bf)),
            }
        )
    return in_maps


def kernel(inX, W_qkv, b_qkv, W_out, b_out):
    global LAST_EXEC_NS, LAST_RESULTS
    inX = np.asarray(inX, dtype=np.float32)
    W_qkv = np.asarray(W_qkv, dtype=np.float32)
    b_qkv = np.asarray(b_qkv, dtype=np.float32)
    W_out = np.asarray(W_out, dtype=np.float32)
    b_out = np.asarray(b_out, dtype=np.float32)

    nc = _get_nc()
    in_maps = make_in_maps(inX, W_qkv, b_qkv, W_out)

    kwargs = {}
    if PROFILE:
        kwargs = {"trace": True, "trace_cores": [0]}
    res = bass_utils.run_bass_kernel_spmd(
        nc, in_maps, core_ids=list(range(NCORES)), **kwargs
    )
    LAST_EXEC_NS = res.exec_time_ns
    LAST_RESULTS = res

    bias_full = (b_out + b_qkv[2 * E : 3 * E] @ W_out).astype(np.float32)
    out = np.empty((B, S, E), dtype=np.float32)
    for b in range(B):
        acc = res.results[HG * b + 0]["out"].astype(np.float64)
        for hg in range(1, HG):
            acc += res.results[HG * b + hg]["out"]
        out[b] = (acc + bias_full).astype(np.float32)
    return out


# revision 27
# speedup vs baseline: 1.1998x; 1.1998x over previous
"""Causal self-attention (B=2, S=2048, E=2048, H=16) on 8 TRN2 NeuronCores.

Sharding: 2-way batch x 4-way head-group tensor parallel.
Core c handles batch c//4 and heads [4*(c%4), 4*(c%4)+4).

Single-pass bf16 design (the rel-err gate is 2e-2; bf16 lands ~3e-3):
  phase 1: QKV projection from host-pretransposed X^T (bf16), feature-major
           qT/kT/vT [128hd, S] bf16 kept in SBUF (no DRAM staging).
           SCALE is folded into Wq/bq on the host.
  phase 2: per head: k-major scores (stationary kT block, moving qT) ->
           transposed-causal mask -> exp to bf16 expPT (k-major, which is
           exactly the PV moving layout: no P transposes). Row sums via a
           ones-column matmul accumulated alongside PV; normalization is
           applied to the (small) PV output, with the per-q reciprocal
           broadcast across partitions by a tiny K=1 matmul.
  phase 3: out projection: stationary attO blocks (feature-major), moving
           W_out rows; fp32 partials to DRAM.

Host side: per batch X^T in bf16 (shared by 4 cores), per head-group W
slices in bf16; sum the 4 head-group partials per batch and add
(b_out + b_v @ W_out) once (softmax rows sum to 1, so the v-bias
contribution is a constant row vector).
"""

from contextlib import ExitStack

import ml_dtypes
import numpy as np

import concourse.bass as bass
import concourse.tile as tile
from concourse import bacc, bass_utils, mybir
from concourse.masks import make_identity

FP = mybir.dt.float32
BF = mybir.dt.bfloat16
AF = mybir.ActivationFunctionType

B = 2
S = 2048
E = 2048
H = 16
HD = 128
NCORES = 8
HG = 4  # head-group axis (tensor parallel)
H_LOC = H // HG  # 4 heads per core
FLOC = H_LOC * HD  # 512 local features per q/k/v
SCALE = 1.0 / float(np.sqrt(HD))
NEG = -1.0e30

PROFILE = False
LAST_EXEC_NS = None
LAST_RESULTS = None


def _emit(nc, S=S, E=E):
    NB = S // 128  # token blocks
    EB = E // 128  # embed blocks
    NC = S // 512  # 512-wide q chunks

    xT = nc.dram_tensor("xT", [E, S], BF, kind="ExternalInput").ap()
    wq = nc.dram_tensor("wq", [E, FLOC], BF, kind="ExternalInput").ap()
    wk = nc.dram_tensor("wk", [E, FLOC], BF, kind="ExternalInput").ap()
    wv = nc.dram_tensor("wv", [E, FLOC], BF, kind="ExternalInput").ap()
    bqs = nc.dram_tensor("bqs", [FLOC, 1], FP, kind="ExternalInput").ap()  # *SCALE
    bk = nc.dram_tensor("bk", [FLOC, 1], FP, kind="ExternalInput").ap()
    wo = nc.dram_tensor("wo", [FLOC, E], BF, kind="ExternalInput").ap()
    out = nc.dram_tensor("out", [S, E], FP, kind="ExternalOutput").ap()

    with tile.TileContext(nc) as tc, ExitStack() as top:
        cst = top.enter_context(tc.tile_pool(name="cst", bufs=1))
        ident_bf = cst.tile([128, 128], BF, name="identbf", tag="identbf")
        make_identity(nc, ident_bf[:])
        # transposed causal mask: keep (0) where q(free) >= k(part), NEG below
        maskT = cst.tile([128, 128], FP, name="maskT", tag="maskT")
        nc.gpsimd.memset(maskT[:], 0.0)
        nc.gpsimd.affine_select(
            out=maskT[:],
            in_=maskT[:],
            compare_op=mybir.AluOpType.is_ge,
            fill=NEG,
            base=0,
            pattern=[[1, 128]],  # +1 * free index
            channel_multiplier=-1,  # -1 * partition index
        )
        # 32 identical ones columns: rowsum matmuls write 32 equal psum rows,
        # so the downstream copy/reciprocal run 32-wide, not single-partition
        ones_col = cst.tile([128, 32], BF, name="onescol", tag="onescol")
        nc.vector.memset(ones_col[:], 1.0)
        # ones rows at base partitions 0/32/64 for the bcast matmuls
        ones_row = cst.tile([96, 128], FP, name="onesrow", tag="onesrow")
        nc.vector.memset(ones_row[:], 1.0)
        bq_sb = cst.tile([128, H_LOC], FP, name="bq", tag="bq")
        bk_sb = cst.tile([128, H_LOC], FP, name="bk", tag="bk")
        for f in range(H_LOC):
            nc.sync.dma_start(bq_sb[:, f : f + 1], bqs[128 * f : 128 * (f + 1), :])
            nc.sync.dma_start(bk_sb[:, f : f + 1], bk[128 * f : 128 * (f + 1), :])

        # feature-major q/k/v, bf16, SBUF-resident
        qkv_pool = top.enter_context(tc.tile_pool(name="qkvT", bufs=1))
        qT = [qkv_pool.tile([128, S], BF, name=f"qT{h}", tag=f"qT{h}") for h in range(H_LOC)]
        kT = [qkv_pool.tile([128, S], BF, name=f"kT{h}", tag=f"kT{h}") for h in range(H_LOC)]
        vT = [qkv_pool.tile([128, S], BF, name=f"vT{h}", tag=f"vT{h}") for h in range(H_LOC)]
        qkvT = [qT, kT, vT]
        # feature-major attention outputs, bf16, SBUF-resident
        att_pool = top.enter_context(tc.tile_pool(name="att", bufs=1))
        attO = [
            att_pool.tile([128, S], BF, name=f"attO{h}", tag=f"attO{h}")
            for h in range(H_LOC)
        ]

        # ---------------- phase 1: QKV projection ----------------
        with ExitStack() as ph, nc.named_scope("proj"):
            xt_pool = ph.enter_context(tc.tile_pool(name="xt", bufs=1))
            wpool = ph.enter_context(tc.tile_pool(name="w", bufs=1))
            ps_qkv = ph.enter_context(tc.tile_pool(name="ps_qkv", bufs=8, space="PSUM"))
            xts = [
                xt_pool.tile([128, S], BF, name=f"xt{e}", tag=f"xt{e}")
                for e in range(EB)
            ]
            wsb = [
                wpool.tile([128, EB * FLOC], BF, name=f"wsb{wi}", tag=f"wsb{wi}")
                for wi in range(3)
            ]
            # DMA order = first-use order: wq/xt slabs interleaved (the first
            # accumulation chain consumes them e-ascending), then wk, wv
            for e in range(EB):
                nc.sync.dma_start(
                    wsb[0][:, FLOC * e : FLOC * (e + 1)],
                    wq[128 * e : 128 * (e + 1), :],
                )
                nc.sync.dma_start(xts[e][:], xT[128 * e : 128 * (e + 1), :])
            for wi, wsrc in ((1, wk), (2, wv)):
                for e in range(EB):
                    nc.sync.dma_start(
                        wsb[wi][:, FLOC * e : FLOC * (e + 1)],
                        wsrc[128 * e : 128 * (e + 1), :],
                    )

            for which in range(3):
                for h in range(H_LOC):
                    psums = [
                        ps_qkv.tile([128, 512], FP, name="psq", tag="psq")
                        for _ in range(NC)
                    ]
                    for e in range(EB):
                        wt = wsb[which][
                            :, FLOC * e + 128 * h : FLOC * e + 128 * (h + 1)
                        ]
                        first = e == 0
                        last = e == EB - 1
                        for sc in range(NC):
                            nc.tensor.matmul(
                                psums[sc][:],
                                wt,
                                xts[e][:, 512 * sc : 512 * (sc + 1)],
                                start=first,
                                stop=last,
                            )
                    dst = qkvT[which][h]
                    for sc in range(NC):
                        sl = slice(512 * sc, 512 * (sc + 1))
                        if which == 0:
                            nc.vector.tensor_scalar_add(
                                dst[:, sl], psums[sc][:], bq_sb[:, h : h + 1]
                            )
                        elif which == 1:
                            nc.vector.tensor_scalar_add(
                                dst[:, sl], psums[sc][:], bk_sb[:, h : h + 1]
                            )
                        else:
                            nc.scalar.activation(dst[:, sl], psums[sc][:], AF.Copy)

        # ---------------- phase 2: attention per head ----------------
        with ExitStack() as ah:
            vsb_pool = ah.enter_context(tc.tile_pool(name="vsb", bufs=2))
            ept_pool = ah.enter_context(tc.tile_pool(name="ept", bufs=2))
            rsb_pool = ah.enter_context(tc.tile_pool(name="rsb", bufs=2))
            bcs_pool = ah.enter_context(tc.tile_pool(name="bcs", bufs=2))
            ps_sc = ah.enter_context(tc.tile_pool(name="ps_sc", bufs=2, space="PSUM"))
            ps_pv = ah.enter_context(tc.tile_pool(name="ps_pv", bufs=4, space="PSUM"))
            ps_rs = ah.enter_context(tc.tile_pool(name="ps_rs", bufs=2, space="PSUM"))
            # PE matmul outputs must start at partition 0/32/64: spread the
            # 4 per-chunk rowsum rows over two banks at those offsets
            RS_POS = [(0, 0), (0, 32), (0, 64), (1, 0)]

            for h in range(H_LOC):
                with nc.named_scope(f"attn{h}"):
                    # V -> token-major [128 tok, hd] blocks along S
                    vsb = vsb_pool.tile([128, S], BF, name="vsb", tag="vsb")
                    for mg in range(NB // 4):
                        pv = ps_sc.tile([128, 512], BF, name="pst", tag="scr")
                        for m in range(4):
                            i = 4 * mg + m
                            nc.tensor.transpose(
                                pv[:, 128 * m : 128 * (m + 1)],
                                vT[h][:, 128 * i : 128 * (i + 1)],
                                ident_bf[:],
                            )
                        nc.scalar.activation(
                            vsb[:, 512 * mg : 512 * (mg + 1)], pv[:], AF.Copy
                        )

                    pv_ps = [
                        ps_pv.tile([128, 512], FP, name="pspv", tag="pspv")
                        for _ in range(NC)
                    ]
                    rs_ps = [
                        ps_rs.tile([128, 512], FP, name="psrs", tag="psrs")
                        for _ in range(2)
                    ]

                    # software pipeline: rs/PV for kb-1 run while exp(kb) is
                    # still on the scalar engine, so the PE never waits on exp
                    epts = [None] * NB
                    for kb in range(NB + 1):
                        if kb < NB:
                            k0 = 128 * kb
                            cmin = kb // 4
                            ept = ept_pool.tile([128, S], BF, name="ept", tag="ept")
                            epts[kb] = ept
                            # scores^T (k-major) + mask + exp, per 512-q chunk
                            for c in range(cmin, NC):
                                qlo = max(0, k0 - 512 * c)
                                q0 = 512 * c
                                scp = ps_sc.tile(
                                    [128, 512], FP, name="pssc", tag="scr"
                                )
                                nc.tensor.matmul(
                                    scp[:, qlo:512],
                                    kT[h][:, k0 : k0 + 128],
                                    qT[h][:, q0 + qlo : q0 + 512],
                                    start=True,
                                    stop=True,
                                )
                                if c == cmin:
                                    nc.vector.tensor_add(
                                        scp[:, qlo : qlo + 128],
                                        scp[:, qlo : qlo + 128],
                                        maskT[:],
                                    )
                                nc.scalar.activation(
                                    ept[:, q0 + qlo : q0 + 512],
                                    scp[:, qlo:512],
                                    AF.Exp,
                                )
                        if kb == 0:
                            continue
                        pb = kb - 1
                        k0 = 128 * pb
                        cmin = pb // 4
                        ept = epts[pb]
                        # row sums (ones-column matmul), then PV
                        for c in range(cmin, NC):
                            qlo = max(0, k0 - 512 * c)
                            q0 = 512 * c
                            rt, rp = RS_POS[c]
                            nc.tensor.matmul(
                                rs_ps[rt][rp : rp + 32, qlo:512],
                                ones_col[:],
                                ept[:, q0 + qlo : q0 + 512],
                                start=(pb == 0),
                                stop=(pb == 4 * c + 3),
                            )
                        for c in range(cmin, NC):
                            qlo = max(0, k0 - 512 * c)
                            q0 = 512 * c
                            nc.tensor.matmul(
                                pv_ps[c][:, qlo:512],
                                vsb[:, k0 : k0 + 128],
                                ept[:, q0 + qlo : q0 + 512],
                                start=(pb == 0),
                                stop=(pb == 4 * c + 3),
                            )
                            if pb == 4 * c + 3:
                                nc.scalar.activation(
                                    attO[h][:, q0 : q0 + 512],
                                    pv_ps[c][:],
                                    AF.Copy,
                                )

                    # normalize lazily: attO already holds unnormalized PV;
                    # recip rowsums (2 wide calls), PE-broadcast, in-place mul
                    rs_sb = rsb_pool.tile([96, 1024], FP, name="rssb", tag="rssb")
                    nc.vector.reciprocal(rs_sb[:, 0:512], rs_ps[0][0:96, :])
                    nc.vector.reciprocal(
                        rs_sb[0:32, 512:1024], rs_ps[1][0:32, :]
                    )
                    for c in range(NC):
                        rt, rp = RS_POS[c]
                        bc_ps = ps_rs.tile([128, 512], FP, name="psbc", tag="psrs")
                        nc.tensor.matmul(
                            bc_ps[:],
                            ones_row[rp : rp + 1, :],
                            rs_sb[rp : rp + 1, 512 * rt : 512 * (rt + 1)],
                            start=True,
                            stop=True,
                        )
                        bc_sb = bcs_pool.tile([128, 512], FP, name="bcsb", tag="bcsb")
                        nc.scalar.activation(bc_sb[:], bc_ps[:], AF.Copy)
                        nc.vector.tensor_mul(
                            attO[h][:, 512 * c : 512 * (c + 1)],
                            attO[h][:, 512 * c : 512 * (c + 1)],
                            bc_sb[:],
                        )

        # ---------------- phase 3: output projection ----------------
        with ExitStack() as ph, nc.named_scope("outproj"):
            wo_pool = ph.enter_context(tc.tile_pool(name="wo", bufs=1))
            ostg = ph.enter_context(tc.tile_pool(name="ostg", bufs=4))
            ps_out = ph.enter_context(tc.tile_pool(name="ps_out", bufs=8, space="PSUM"))
            wos = []
            for h in range(H_LOC):
                wt = wo_pool.tile([128, E], BF, name=f"wo{h}", tag=f"wo{h}")
                nc.sync.dma_start(wt[:], wo[128 * h : 128 * (h + 1), :])
                wos.append(wt)
            nec = E // 512
            for i in range(NB):
                psums = [
                    ps_out.tile([128, 512], FP, name="pso", tag="pso")
                    for _ in range(nec)
                ]
                for h in range(H_LOC):
                    ah_blk = attO[h][:, 128 * i : 128 * (i + 1)]
                    for c in range(nec):
                        nc.tensor.matmul(
                            psums[c][:],
                            ah_blk,
                            wos[h][:, 512 * c : 512 * (c + 1)],
                            start=(h == 0),
                            stop=(h == H_LOC - 1),
                        )
                for c in range(nec):
                    ot = ostg.tile([128, 512], FP, name="ostg", tag="ostg")
                    if c % 2 == 0:
                        nc.vector.tensor_copy(ot[:], psums[c][:])
                    else:
                        nc.scalar.activation(ot[:], psums[c][:], AF.Copy)
                    nc.sync.dma_start(
                        out[128 * i : 128 * (i + 1), 512 * c : 512 * (c + 1)],
                        ot[:],
                    )


_NC_CACHE = None


def _get_nc():
    global _NC_CACHE
    if _NC_CACHE is None:
        nc = bacc.Bacc(
            "TRN2",
            target_bir_lowering=False,
            debug=False,
            num_devices=1,
            enable_asserts=False,
        )
        _emit(nc)
        nc.compile()
        _NC_CACHE = nc
    return _NC_CACHE


def make_in_maps(inX, W_qkv, b_qkv, W_out):
    bf = ml_dtypes.bfloat16
    xTs = [np.ascontiguousarray(inX[b].T.astype(bf)) for b in range(B)]
    in_maps = []
    for c in range(NCORES):
        b = c // HG
        hg = c % HG
        sl = slice(FLOC * hg, FLOC * (hg + 1))
        in_maps.append(
            {
                "xT": xTs[b],
                "wq": np.ascontiguousarray(
                    (W_qkv[:, 0:E][:, sl] * SCALE).astype(bf)
                ),
                "wk": np.ascontiguousarray(W_qkv[:, E : 2 * E][:, sl].astype(bf)),
                "wv": np.ascontiguousarray(W_qkv[:, 2 * E : 3 * E][:, sl].astype(bf)),
                "bqs": np.ascontiguousarray(
                    (b_qkv[0:E][sl] * SCALE).reshape(FLOC, 1).astype(np.float32)
                ),
                "bk": np.ascontiguousarray(
                    b_qkv[E : 2 * E][sl].reshape(FLOC, 1).astype(np.float32)
                ),
                "wo": np.ascontiguousarray(W_out[sl, :].astype(bf)),
            }
        )
    return in_maps


def kernel(inX, W_qkv, b_qkv, W_out, b_out):
    global LAST_EXEC_NS, LAST_RESULTS
    inX = np.asarray(inX, dtype=np.float32)
    W_qkv = np.asarray(W_qkv, dtype=np.float32)
    b_qkv = np.asarray(b_qkv, dtype=np.float32)
    W_out = np.asarray(W_out, dtype=np.float32)
    b_out = np.asarray(b_out, dtype=np.float32)

    nc = _get_nc()
    in_maps = make_in_maps(inX, W_qkv, b_qkv, W_out)

    kwargs = {}
    if PROFILE:
        kwargs = {"trace": True, "trace_cores": [0]}
    res = bass_utils.run_bass_kernel_spmd(
        nc, in_maps, core_ids=list(range(NCORES)), **kwargs
    )
    LAST_EXEC_NS = res.exec_time_ns
    LAST_RESULTS = res

    bias_full = (b_out + b_qkv[2 * E : 3 * E] @ W_out).astype(np.float32)
    out = np.empty((B, S, E), dtype=np.float32)
    for b in range(B):
        acc = res.results[HG * b + 0]["out"].astype(np.float64)
        for hg in range(1, HG):
            acc += res.results[HG * b + hg]["out"]
        out[b] = (acc + bias_full).astype(np.float32)
    return out


# revision 28
# speedup vs baseline: 1.2135x; 1.0114x over previous
"""Causal self-attention (B=2, S=2048, E=2048, H=16) on 8 TRN2 NeuronCores.

Sharding: 2-way batch x 4-way head-group tensor parallel.
Core c handles batch c//4 and heads [4*(c%4), 4*(c%4)+4).

Single-pass bf16 design (the rel-err gate is 2e-2; bf16 lands ~3e-3):
  phase 1: QKV projection from host-pretransposed X^T (bf16), feature-major
           qT/kT/vT [128hd, S] bf16 kept in SBUF (no DRAM staging).
           SCALE is folded into Wq/bq on the host.
  phase 2: per head: k-major scores (stationary kT block, moving qT) ->
           transposed-causal mask -> exp to bf16 expPT (k-major, which is
           exactly the PV moving layout: no P transposes). Row sums via a
           ones-column matmul accumulated alongside PV; normalization is
           applied to the (small) PV output, with the per-q reciprocal
           broadcast across partitions by a tiny K=1 matmul.
  phase 3: out projection: stationary attO blocks (feature-major), moving
           W_out rows; fp32 partials to DRAM.

Host side: per batch X^T in bf16 (shared by 4 cores), per head-group W
slices in bf16; sum the 4 head-group partials per batch and add
(b_out + b_v @ W_out) once (softmax rows sum to 1, so the v-bias
contribution is a constant row vector).
"""

from contextlib import ExitStack

import ml_dtypes
import numpy as np

import concourse.bass as bass
import concourse.tile as tile
from concourse import bacc, bass_utils, mybir
from concourse.masks import make_identity

FP = mybir.dt.float32
BF = mybir.dt.bfloat16
AF = mybir.ActivationFunctionType

B = 2
S = 2048
E = 2048
H = 16
HD = 128
NCORES = 8
HG = 4  # head-group axis (tensor parallel)
H_LOC = H // HG  # 4 heads per core
FLOC = H_LOC * HD  # 512 local features per q/k/v
SCALE = 1.0 / float(np.sqrt(HD))
NEG = -1.0e30

PROFILE = False
LAST_EXEC_NS = None
LAST_RESULTS = None


def _emit(nc, S=S, E=E):
    NB = S // 128  # token blocks
    EB = E // 128  # embed blocks
    NC = S // 512  # 512-wide q chunks

    xT = nc.dram_tensor("xT", [E, S], BF, kind="ExternalInput").ap()
    wq = nc.dram_tensor("wq", [E, FLOC], BF, kind="ExternalInput").ap()
    wk = nc.dram_tensor("wk", [E, FLOC], BF, kind="ExternalInput").ap()
    wv = nc.dram_tensor("wv", [E, FLOC], BF, kind="ExternalInput").ap()
    bqs = nc.dram_tensor("bqs", [FLOC, 1], FP, kind="ExternalInput").ap()  # *SCALE
    bk = nc.dram_tensor("bk", [FLOC, 1], FP, kind="ExternalInput").ap()
    wo = nc.dram_tensor("wo", [FLOC, E], BF, kind="ExternalInput").ap()
    out = nc.dram_tensor("out", [S, E], FP, kind="ExternalOutput").ap()

    with tile.TileContext(nc) as tc, ExitStack() as top:
        cst = top.enter_context(tc.tile_pool(name="cst", bufs=1))
        ident_bf = cst.tile([128, 128], BF, name="identbf", tag="identbf")
        make_identity(nc, ident_bf[:])
        # transposed causal mask: keep (0) where q(free) >= k(part), NEG below
        maskT = cst.tile([128, 128], FP, name="maskT", tag="maskT")
        nc.gpsimd.memset(maskT[:], 0.0)
        nc.gpsimd.affine_select(
            out=maskT[:],
            in_=maskT[:],
            compare_op=mybir.AluOpType.is_ge,
            fill=NEG,
            base=0,
            pattern=[[1, 128]],  # +1 * free index
            channel_multiplier=-1,  # -1 * partition index
        )
        # 32 identical ones columns: rowsum matmuls write 32 equal psum rows,
        # so the downstream copy/reciprocal run 32-wide, not single-partition
        ones_col = cst.tile([128, 32], BF, name="onescol", tag="onescol")
        nc.vector.memset(ones_col[:], 1.0)
        # ones rows at base partitions 0/32/64 for the bcast matmuls
        ones_row = cst.tile([96, 128], FP, name="onesrow", tag="onesrow")
        nc.vector.memset(ones_row[:], 1.0)
        bq_sb = cst.tile([128, H_LOC], FP, name="bq", tag="bq")
        bk_sb = cst.tile([128, H_LOC], FP, name="bk", tag="bk")
        for f in range(H_LOC):
            nc.sync.dma_start(bq_sb[:, f : f + 1], bqs[128 * f : 128 * (f + 1), :])
            nc.sync.dma_start(bk_sb[:, f : f + 1], bk[128 * f : 128 * (f + 1), :])

        # feature-major q/k/v, bf16, SBUF-resident
        qkv_pool = top.enter_context(tc.tile_pool(name="qkvT", bufs=1))
        qT = [qkv_pool.tile([128, S], BF, name=f"qT{h}", tag=f"qT{h}") for h in range(H_LOC)]
        kT = [qkv_pool.tile([128, S], BF, name=f"kT{h}", tag=f"kT{h}") for h in range(H_LOC)]
        vT = [qkv_pool.tile([128, S], BF, name=f"vT{h}", tag=f"vT{h}") for h in range(H_LOC)]
        qkvT = [qT, kT, vT]
        # feature-major attention outputs, bf16, SBUF-resident
        att_pool = top.enter_context(tc.tile_pool(name="att", bufs=1))
        attO = [
            att_pool.tile([128, S], BF, name=f"attO{h}", tag=f"attO{h}")
            for h in range(H_LOC)
        ]

        # ---------------- phase 1: QKV projection ----------------
        with ExitStack() as ph, nc.named_scope("proj"):
            xt_pool = ph.enter_context(tc.tile_pool(name="xt", bufs=1))
            wpool = ph.enter_context(tc.tile_pool(name="w", bufs=1))
            ps_qkv = ph.enter_context(tc.tile_pool(name="ps_qkv", bufs=8, space="PSUM"))
            xts = [
                xt_pool.tile([128, S], BF, name=f"xt{e}", tag=f"xt{e}")
                for e in range(EB)
            ]
            wsb = [
                wpool.tile([128, EB * FLOC], BF, name=f"wsb{wi}", tag=f"wsb{wi}")
                for wi in range(3)
            ]
            # DMA order = first-use order: wq/xt slabs interleaved (the first
            # accumulation chain consumes them e-ascending), then wk, wv
            for e in range(EB):
                nc.sync.dma_start(
                    wsb[0][:, FLOC * e : FLOC * (e + 1)],
                    wq[128 * e : 128 * (e + 1), :],
                )
                nc.sync.dma_start(xts[e][:], xT[128 * e : 128 * (e + 1), :])
            for wi, wsrc in ((1, wk), (2, wv)):
                for e in range(EB):
                    nc.sync.dma_start(
                        wsb[wi][:, FLOC * e : FLOC * (e + 1)],
                        wsrc[128 * e : 128 * (e + 1), :],
                    )

            for which in range(3):
                for h in range(H_LOC):
                    psums = [
                        ps_qkv.tile([128, 512], FP, name="psq", tag="psq")
                        for _ in range(NC)
                    ]
                    for e in range(EB):
                        wt = wsb[which][
                            :, FLOC * e + 128 * h : FLOC * e + 128 * (h + 1)
                        ]
                        first = e == 0
                        last = e == EB - 1
                        for sc in range(NC):
                            nc.tensor.matmul(
                                psums[sc][:],
                                wt,
                                xts[e][:, 512 * sc : 512 * (sc + 1)],
                                start=first,
                                stop=last,
                            )
                    dst = qkvT[which][h]
                    for sc in range(NC):
                        sl = slice(512 * sc, 512 * (sc + 1))
                        if which == 0:
                            nc.vector.tensor_scalar_add(
                                dst[:, sl], psums[sc][:], bq_sb[:, h : h + 1]
                            )
                        elif which == 1:
                            nc.vector.tensor_scalar_add(
                                dst[:, sl], psums[sc][:], bk_sb[:, h : h + 1]
                            )
                        else:
                            nc.scalar.activation(dst[:, sl], psums[sc][:], AF.Copy)

        # ---------------- phase 2: attention per head ----------------
        with ExitStack() as ah:
            vsb_pool = ah.enter_context(tc.tile_pool(name="vsb", bufs=2))
            ept_pool = ah.enter_context(tc.tile_pool(name="ept", bufs=2))
            rsb_pool = ah.enter_context(tc.tile_pool(name="rsb", bufs=2))
            bcs_pool = ah.enter_context(tc.tile_pool(name="bcs", bufs=2))
            ps_sc = ah.enter_context(tc.tile_pool(name="ps_sc", bufs=2, space="PSUM"))
            ps_pv = ah.enter_context(tc.tile_pool(name="ps_pv", bufs=4, space="PSUM"))
            ps_rs = ah.enter_context(tc.tile_pool(name="ps_rs", bufs=2, space="PSUM"))
            # PE matmul outputs must start at partition 0/32/64: spread the
            # 4 per-chunk rowsum rows over two banks at those offsets
            RS_POS = [(0, 0), (0, 32), (0, 64), (1, 0)]

            for h in range(H_LOC):
                with nc.named_scope(f"attn{h}"):
                    # V -> token-major [128 tok, hd] blocks along S
                    vsb = vsb_pool.tile([128, S], BF, name="vsb", tag="vsb")
                    for mg in range(NB // 4):
                        pv = ps_sc.tile([128, 512], BF, name="pst", tag="scr")
                        for m in range(4):
                            i = 4 * mg + m
                            nc.tensor.transpose(
                                pv[:, 128 * m : 128 * (m + 1)],
                                vT[h][:, 128 * i : 128 * (i + 1)],
                                ident_bf[:],
                            )
                        nc.vector.tensor_copy(
                            vsb[:, 512 * mg : 512 * (mg + 1)], pv[:]
                        )

                    pv_ps = [
                        ps_pv.tile([128, 512], FP, name="pspv", tag="pspv")
                        for _ in range(NC)
                    ]
                    rs_ps = [
                        ps_rs.tile([128, 512], FP, name="psrs", tag="psrs")
                        for _ in range(2)
                    ]

                    # software pipeline: rs/PV for kb-1 run while exp(kb) is
                    # still on the scalar engine, so the PE never waits on exp
                    epts = [None] * NB
                    for kb in range(NB + 1):
                        if kb < NB:
                            k0 = 128 * kb
                            cmin = kb // 4
                            ept = ept_pool.tile([128, S], BF, name="ept", tag="ept")
                            epts[kb] = ept
                            # scores^T (k-major) + mask + exp, per 512-q chunk
                            for c in range(cmin, NC):
                                qlo = max(0, k0 - 512 * c)
                                q0 = 512 * c
                                scp = ps_sc.tile(
                                    [128, 512], FP, name="pssc", tag="scr"
                                )
                                nc.tensor.matmul(
                                    scp[:, qlo:512],
                                    kT[h][:, k0 : k0 + 128],
                                    qT[h][:, q0 + qlo : q0 + 512],
                                    start=True,
                                    stop=True,
                                )
                                if c == cmin:
                                    nc.vector.tensor_add(
                                        scp[:, qlo : qlo + 128],
                                        scp[:, qlo : qlo + 128],
                                        maskT[:],
                                    )
                                nc.scalar.activation(
                                    ept[:, q0 + qlo : q0 + 512],
                                    scp[:, qlo:512],
                                    AF.Exp,
                                )
                        if kb == 0:
                            continue
                        pb = kb - 1
                        k0 = 128 * pb
                        cmin = pb // 4
                        ept = epts[pb]
                        # row sums (ones-column matmul), then PV
                        for c in range(cmin, NC):
                            qlo = max(0, k0 - 512 * c)
                            q0 = 512 * c
                            rt, rp = RS_POS[c]
                            nc.tensor.matmul(
                                rs_ps[rt][rp : rp + 32, qlo:512],
                                ones_col[:],
                                ept[:, q0 + qlo : q0 + 512],
                                start=(pb == 0),
                                stop=(pb == 4 * c + 3),
                            )
                        for c in range(cmin, NC):
                            qlo = max(0, k0 - 512 * c)
                            q0 = 512 * c
                            nc.tensor.matmul(
                                pv_ps[c][:, qlo:512],
                                vsb[:, k0 : k0 + 128],
                                ept[:, q0 + qlo : q0 + 512],
                                start=(pb == 0),
                                stop=(pb == 4 * c + 3),
                            )
                            if pb == 4 * c + 3:
                                nc.vector.tensor_copy(
                                    attO[h][:, q0 : q0 + 512], pv_ps[c][:]
                                )

                    # normalize lazily: attO already holds unnormalized PV;
                    # recip rowsums (2 wide calls), PE-broadcast, in-place mul
                    rs_sb = rsb_pool.tile([96, 1024], FP, name="rssb", tag="rssb")
                    nc.vector.reciprocal(rs_sb[:, 0:512], rs_ps[0][0:96, :])
                    nc.vector.reciprocal(
                        rs_sb[0:32, 512:1024], rs_ps[1][0:32, :]
                    )
                    for c in range(NC):
                        rt, rp = RS_POS[c]
                        bc_ps = ps_rs.tile([128, 512], FP, name="psbc", tag="psrs")
                        nc.tensor.matmul(
                            bc_ps[:],
                            ones_row[rp : rp + 1, :],
                            rs_sb[rp : rp + 1, 512 * rt : 512 * (rt + 1)],
                            start=True,
                            stop=True,
                        )
                        bc_sb = bcs_pool.tile([128, 512], FP, name="bcsb", tag="bcsb")
                        nc.vector.tensor_copy(bc_sb[:], bc_ps[:])
                        nc.vector.tensor_mul(
                            attO[h][:, 512 * c : 512 * (c + 1)],
                            attO[h][:, 512 * c : 512 * (c + 1)],
                            bc_sb[:],
                        )

        # ---------------- phase 3: output projection ----------------
        with ExitStack() as ph, nc.named_scope("outproj"):
            wo_pool = ph.enter_context(tc.tile_pool(name="wo", bufs=1))
            ostg = ph.enter_context(tc.tile_pool(name="ostg", bufs=4))
            ps_out = ph.enter_context(tc.tile_pool(name="ps_out", bufs=8, space="PSUM"))
            wos = []
            for h in range(H_LOC):
                wt = wo_pool.tile([128, E], BF, name=f"wo{h}", tag=f"wo{h}")
                nc.sync.dma_start(wt[:], wo[128 * h : 128 * (h + 1), :])
                wos.append(wt)
            nec = E // 512
            for i in range(NB):
                psums = [
                    ps_out.tile([128, 512], FP, name="pso", tag="pso")
                    for _ in range(nec)
                ]
                for h in range(H_LOC):
                    ah_blk = attO[h][:, 128 * i : 128 * (i + 1)]
                    for c in range(nec):
                        nc.tensor.matmul(
                            psums[c][:],
                            ah_blk,
                            wos[h][:, 512 * c : 512 * (c + 1)],
                            start=(h == 0),
                            stop=(h == H_LOC - 1),
                        )
                for c in range(nec):
                    ot = ostg.tile([128, 512], FP, name="ostg", tag="ostg")
                    if c % 2 == 0:
                        nc.vector.tensor_copy(ot[:], psums[c][:])
                    else:
                        nc.scalar.activation(ot[:], psums[c][:], AF.Copy)
                    dq = nc.sync if c % 2 == 0 else nc.scalar
                    dq.dma_start(
                        out[128 * i : 128 * (i + 1), 512 * c : 512 * (c + 1)],
                        ot[:],
                    )


_NC_CACHE = None


def _get_nc():
    global _NC_CACHE
    if _NC_CACHE is None:
        nc = bacc.Bacc(
            "TRN2",
            target_bir_lowering=False,
            debug=False,
            num_devices=1,
            enable_asserts=False,
        )
        _emit(nc)
        nc.compile()
        _NC_CACHE = nc
    return _NC_CACHE


def make_in_maps(inX, W_qkv, b_qkv, W_out):
    bf = ml_dtypes.bfloat16
    xTs = [np.ascontiguousarray(inX[b].T.astype(bf)) for b in range(B)]
    in_maps = []
    for c in range(NCORES):
        b = c // HG
        hg = c % HG
        sl = slice(FLOC * hg, FLOC * (hg + 1))
        in_maps.append(
            {
                "xT": xTs[b],
                "wq": np.ascontiguousarray(
                    (W_qkv[:, 0:E][:, sl] * SCALE).astype(bf)
                ),
                "wk": np.ascontiguousarray(W_qkv[:, E : 2 * E][:, sl].astype(bf)),
                "wv": np.ascontiguousarray(W_qkv[:, 2 * E : 3 * E][:, sl].astype(bf)),
                "bqs": np.ascontiguousarray(
                    (b_qkv[0:E][sl] * SCALE).reshape(FLOC, 1).astype(np.float32)
                ),
                "bk": np.ascontiguousarray(
                    b_qkv[E : 2 * E][sl].reshape(FLOC, 1).astype(np.float32)
                ),
                "wo": np.ascontiguousarray(W_out[sl, :].astype(bf)),
            }
        )
    return in_maps


def kernel(inX, W_qkv, b_qkv, W_out, b_out):
    global LAST_EXEC_NS, LAST_RESULTS
    inX = np.asarray(inX, dtype=np.float32)
    W_qkv = np.asarray(W_qkv, dtype=np.float32)
    b_qkv = np.asarray(b_qkv, dtype=np.float32)
    W_out = np.asarray(W_out, dtype=np.float32)
    b_out = np.asarray(b_out, dtype=np.float32)

    nc = _get_nc()
    in_maps = make_in_maps(inX, W_qkv, b_qkv, W_out)

    kwargs = {}
    if PROFILE:
        kwargs = {"trace": True, "trace_cores": [0]}
    res = bass_utils.run_bass_kernel_spmd(
        nc, in_maps, core_ids=list(range(NCORES)), **kwargs
    )
    LAST_EXEC_NS = res.exec_time_ns
    LAST_RESULTS = res

    bias_full = (b_out + b_qkv[2 * E : 3 * E] @ W_out).astype(np.float32)
    out = np.empty((B, S, E), dtype=np.float32)
    for b in range(B):
        acc = res.results[HG * b + 0]["out"].astype(np.float64)
        for hg in range(1, HG):
            acc += res.results[HG * b + hg]["out"]
        out[b] = (acc + bias_full).astype(np.float32)
    return out


# revision 30
# speedup vs baseline: 1.2561x; 1.0351x over previous
"""Causal self-attention (B=2, S=2048, E=2048, H=16) on 8 TRN2 NeuronCores.

Sharding: 2-way batch x 4-way head-group tensor parallel.
Core c handles batch c//4 and heads [4*(c%4), 4*(c%4)+4).

Single-pass bf16 design (the rel-err gate is 2e-2; bf16 lands ~3e-3):
  phase 1: QKV projection from host-pretransposed X^T (bf16), feature-major
           qT/kT/vT [128hd, S] bf16 kept in SBUF (no DRAM staging).
           SCALE is folded into Wq/bq on the host.
  phase 2: per head: k-major scores (stationary kT block, moving qT) ->
           transposed-causal mask -> exp to bf16 expPT (k-major, which is
           exactly the PV moving layout: no P transposes). Row sums via a
           ones-column matmul accumulated alongside PV; normalization is
           applied to the (small) PV output, with the per-q reciprocal
           broadcast across partitions by a tiny K=1 matmul.
  phase 3: out projection: stationary attO blocks (feature-major), moving
           W_out rows; fp32 partials to DRAM.

Host side: per batch X^T in bf16 (shared by 4 cores), per head-group W
slices in bf16; sum the 4 head-group partials per batch and add
(b_out + b_v @ W_out) once (softmax rows sum to 1, so the v-bias
contribution is a constant row vector).
"""

from contextlib import ExitStack

import ml_dtypes
import numpy as np

import concourse.bass as bass
import concourse.tile as tile
from concourse import bacc, bass_utils, mybir
from concourse.masks import make_identity

FP = mybir.dt.float32
BF = mybir.dt.bfloat16
AF = mybir.ActivationFunctionType

B = 2
S = 2048
E = 2048
H = 16
HD = 128
NCORES = 8
HG = 4  # head-group axis (tensor parallel)
H_LOC = H // HG  # 4 heads per core
FLOC = H_LOC * HD  # 512 local features per q/k/v
SCALE = 1.0 / float(np.sqrt(HD))
NEG = -1.0e30

PROFILE = False
LAST_EXEC_NS = None
LAST_RESULTS = None


def _emit(nc, S=S, E=E):
    NB = S // 128  # token blocks
    EB = E // 128  # embed blocks
    NC = S // 512  # 512-wide q chunks

    xT = nc.dram_tensor("xT", [E, S], BF, kind="ExternalInput").ap()
    wq = nc.dram_tensor("wq", [E, FLOC], BF, kind="ExternalInput").ap()
    wk = nc.dram_tensor("wk", [E, FLOC], BF, kind="ExternalInput").ap()
    wv = nc.dram_tensor("wv", [E, FLOC], BF, kind="ExternalInput").ap()
    bqs = nc.dram_tensor("bqs", [FLOC, 1], FP, kind="ExternalInput").ap()  # *SCALE
    bk = nc.dram_tensor("bk", [FLOC, 1], FP, kind="ExternalInput").ap()
    wo = nc.dram_tensor("wo", [FLOC, E], BF, kind="ExternalInput").ap()
    out = nc.dram_tensor("out", [S, E], BF, kind="ExternalOutput").ap()

    with tile.TileContext(nc) as tc, ExitStack() as top:
        cst = top.enter_context(tc.tile_pool(name="cst", bufs=1))
        ident_bf = cst.tile([128, 128], BF, name="identbf", tag="identbf")
        make_identity(nc, ident_bf[:])
        # transposed causal mask: keep (0) where q(free) >= k(part), NEG below
        maskT = cst.tile([128, 128], FP, name="maskT", tag="maskT")
        nc.gpsimd.memset(maskT[:], 0.0)
        nc.gpsimd.affine_select(
            out=maskT[:],
            in_=maskT[:],
            compare_op=mybir.AluOpType.is_ge,
            fill=NEG,
            base=0,
            pattern=[[1, 128]],  # +1 * free index
            channel_multiplier=-1,  # -1 * partition index
        )
        # 32 identical ones columns: rowsum matmuls write 32 equal psum rows,
        # so the downstream copy/reciprocal run 32-wide, not single-partition
        ones_col = cst.tile([128, 32], BF, name="onescol", tag="onescol")
        nc.vector.memset(ones_col[:], 1.0)
        # ones rows at base partitions 0/32/64 for the bcast matmuls
        ones_row = cst.tile([96, 128], BF, name="onesrow", tag="onesrow")
        nc.vector.memset(ones_row[:], 1.0)
        bq_sb = cst.tile([128, H_LOC], FP, name="bq", tag="bq")
        bk_sb = cst.tile([128, H_LOC], FP, name="bk", tag="bk")
        for f in range(H_LOC):
            nc.sync.dma_start(bq_sb[:, f : f + 1], bqs[128 * f : 128 * (f + 1), :])
            nc.sync.dma_start(bk_sb[:, f : f + 1], bk[128 * f : 128 * (f + 1), :])

        # feature-major q/k/v, bf16, SBUF-resident
        qkv_pool = top.enter_context(tc.tile_pool(name="qkvT", bufs=1))
        qT = [qkv_pool.tile([128, S], BF, name=f"qT{h}", tag=f"qT{h}") for h in range(H_LOC)]
        kT = [qkv_pool.tile([128, S], BF, name=f"kT{h}", tag=f"kT{h}") for h in range(H_LOC)]
        vT = [qkv_pool.tile([128, S], BF, name=f"vT{h}", tag=f"vT{h}") for h in range(H_LOC)]
        qkvT = [qT, kT, vT]
        # feature-major attention outputs, bf16, SBUF-resident
        att_pool = top.enter_context(tc.tile_pool(name="att", bufs=1))
        attO = [
            att_pool.tile([128, S], BF, name=f"attO{h}", tag=f"attO{h}")
            for h in range(H_LOC)
        ]

        # ---------------- phase 1: QKV projection ----------------
        with ExitStack() as ph, nc.named_scope("proj"):
            xt_pool = ph.enter_context(tc.tile_pool(name="xt", bufs=1))
            wpool = ph.enter_context(tc.tile_pool(name="w", bufs=1))
            ps_qkv = ph.enter_context(tc.tile_pool(name="ps_qkv", bufs=8, space="PSUM"))
            xts = [
                xt_pool.tile([128, S], BF, name=f"xt{e}", tag=f"xt{e}")
                for e in range(EB)
            ]
            wsb = [
                wpool.tile([128, EB * FLOC], BF, name=f"wsb{wi}", tag=f"wsb{wi}")
                for wi in range(3)
            ]
            # DMA order = first-use order: wq/xt slabs interleaved (the first
            # accumulation chain consumes them e-ascending), then wk, wv
            for e in range(EB):
                nc.sync.dma_start(
                    wsb[0][:, FLOC * e : FLOC * (e + 1)],
                    wq[128 * e : 128 * (e + 1), :],
                )
                nc.sync.dma_start(xts[e][:], xT[128 * e : 128 * (e + 1), :])
            for wi, wsrc in ((1, wk), (2, wv)):
                for e in range(EB):
                    nc.sync.dma_start(
                        wsb[wi][:, FLOC * e : FLOC * (e + 1)],
                        wsrc[128 * e : 128 * (e + 1), :],
                    )

            for which in range(3):
                for h in range(H_LOC):
                    psums = [
                        ps_qkv.tile([128, 512], FP, name="psq", tag="psq")
                        for _ in range(NC)
                    ]
                    for e in range(EB):
                        wt = wsb[which][
                            :, FLOC * e + 128 * h : FLOC * e + 128 * (h + 1)
                        ]
                        first = e == 0
                        last = e == EB - 1
                        for sc in range(NC):
                            nc.tensor.matmul(
                                psums[sc][:],
                                wt,
                                xts[e][:, 512 * sc : 512 * (sc + 1)],
                                start=first,
                                stop=last,
                            )
                    dst = qkvT[which][h]
                    for sc in range(NC):
                        sl = slice(512 * sc, 512 * (sc + 1))
                        if which == 0:
                            nc.vector.tensor_scalar_add(
                                dst[:, sl], psums[sc][:], bq_sb[:, h : h + 1]
                            )
                        elif which == 1:
                            nc.vector.tensor_scalar_add(
                                dst[:, sl], psums[sc][:], bk_sb[:, h : h + 1]
                            )
                        else:
                            nc.scalar.activation(dst[:, sl], psums[sc][:], AF.Copy)

        # ---------------- phase 2: attention per head ----------------
        with ExitStack() as ah:
            vsb_pool = ah.enter_context(tc.tile_pool(name="vsb", bufs=2))
            ept_pool = ah.enter_context(tc.tile_pool(name="ept", bufs=2))
            rsb_pool = ah.enter_context(tc.tile_pool(name="rsb", bufs=2))
            bcs_pool = ah.enter_context(tc.tile_pool(name="bcs", bufs=2))
            ps_sc = ah.enter_context(tc.tile_pool(name="ps_sc", bufs=2, space="PSUM"))
            ps_pv = ah.enter_context(tc.tile_pool(name="ps_pv", bufs=4, space="PSUM"))
            ps_rs = ah.enter_context(tc.tile_pool(name="ps_rs", bufs=2, space="PSUM"))
            # PE matmul outputs must start at partition 0/32/64: spread the
            # 4 per-chunk rowsum rows over two banks at those offsets
            RS_POS = [(0, 0), (0, 32), (0, 64), (1, 0)]

            for h in range(H_LOC):
                with nc.named_scope(f"attn{h}"):
                    # V -> token-major [128 tok, hd] blocks along S
                    vsb = vsb_pool.tile([128, S], BF, name="vsb", tag="vsb")
                    for mg in range(NB // 4):
                        pv = ps_sc.tile([128, 512], BF, name="pst", tag="scr")
                        for m in range(4):
                            i = 4 * mg + m
                            nc.tensor.transpose(
                                pv[:, 128 * m : 128 * (m + 1)],
                                vT[h][:, 128 * i : 128 * (i + 1)],
                                ident_bf[:],
                            )
                        nc.vector.tensor_copy(
                            vsb[:, 512 * mg : 512 * (mg + 1)], pv[:]
                        )

                    pv_ps = [
                        ps_pv.tile([128, 512], FP, name="pspv", tag="pspv")
                        for _ in range(NC)
                    ]
                    rs_ps = [
                        ps_rs.tile([128, 512], FP, name="psrs", tag="psrs")
                        for _ in range(2)
                    ]

                    # software pipeline: rs/PV for kb-1 run while exp(kb) is
                    # still on the scalar engine, so the PE never waits on exp
                    epts = [None] * NB
                    for kb in range(NB + 1):
                        if kb < NB:
                            k0 = 128 * kb
                            cmin = kb // 4
                            ept = ept_pool.tile([128, S], BF, name="ept", tag="ept")
                            epts[kb] = ept
                            # scores^T (k-major) + mask + exp, per 512-q chunk
                            for c in range(cmin, NC):
                                qlo = max(0, k0 - 512 * c)
                                q0 = 512 * c
                                scp = ps_sc.tile(
                                    [128, 512], FP, name="pssc", tag="scr"
                                )
                                nc.tensor.matmul(
                                    scp[:, qlo:512],
                                    kT[h][:, k0 : k0 + 128],
                                    qT[h][:, q0 + qlo : q0 + 512],
                                    start=True,
                                    stop=True,
                                )
                                if c == cmin:
                                    nc.vector.tensor_add(
                                        scp[:, qlo : qlo + 128],
                                        scp[:, qlo : qlo + 128],
                                        maskT[:],
                                    )
                                nc.scalar.activation(
                                    ept[:, q0 + qlo : q0 + 512],
                                    scp[:, qlo:512],
                                    AF.Exp,
                                )
                        if kb == 0:
                            continue
                        pb = kb - 1
                        k0 = 128 * pb
                        cmin = pb // 4
                        ept = epts[pb]
                        # row sums (ones-column matmul), then PV
                        for c in range(cmin, NC):
                            qlo = max(0, k0 - 512 * c)
                            q0 = 512 * c
                            rt, rp = RS_POS[c]
                            nc.tensor.matmul(
                                rs_ps[rt][rp : rp + 32, qlo:512],
                                ones_col[:],
                                ept[:, q0 + qlo : q0 + 512],
                                start=(pb == 0),
                                stop=(pb == 4 * c + 3),
                            )
                        for c in range(cmin, NC):
                            qlo = max(0, k0 - 512 * c)
                            q0 = 512 * c
                            nc.tensor.matmul(
                                pv_ps[c][:, qlo:512],
                                vsb[:, k0 : k0 + 128],
                                ept[:, q0 + qlo : q0 + 512],
                                start=(pb == 0),
                                stop=(pb == 4 * c + 3),
                            )
                            if pb == 4 * c + 3:
                                nc.vector.tensor_copy(
                                    attO[h][:, q0 : q0 + 512], pv_ps[c][:]
                                )

                    # normalize lazily: attO already holds unnormalized PV;
                    # recip rowsums (2 wide calls), PE-broadcast, in-place mul
                    rs_sb = rsb_pool.tile([96, 1024], BF, name="rssb", tag="rssb")
                    with nc.allow_low_precision(reason="bf16 softmax denom ok"):
                        nc.vector.reciprocal(rs_sb[:, 0:512], rs_ps[0][0:96, :])
                        nc.vector.reciprocal(
                            rs_sb[0:32, 512:1024], rs_ps[1][0:32, :]
                        )
                    for c in range(NC):
                        rt, rp = RS_POS[c]
                        bc_ps = ps_rs.tile([128, 512], FP, name="psbc", tag="psrs")
                        nc.tensor.matmul(
                            bc_ps[:],
                            ones_row[rp : rp + 1, :],
                            rs_sb[rp : rp + 1, 512 * rt : 512 * (rt + 1)],
                            start=True,
                            stop=True,
                        )
                        bc_sb = bcs_pool.tile([128, 512], FP, name="bcsb", tag="bcsb")
                        nc.vector.tensor_copy(bc_sb[:], bc_ps[:])
                        nc.vector.tensor_mul(
                            attO[h][:, 512 * c : 512 * (c + 1)],
                            attO[h][:, 512 * c : 512 * (c + 1)],
                            bc_sb[:],
                        )

        # ---------------- phase 3: output projection ----------------
        with ExitStack() as ph, nc.named_scope("outproj"):
            wo_pool = ph.enter_context(tc.tile_pool(name="wo", bufs=1))
            ostg = ph.enter_context(tc.tile_pool(name="ostg", bufs=4))
            ps_out = ph.enter_context(tc.tile_pool(name="ps_out", bufs=8, space="PSUM"))
            wos = []
            for h in range(H_LOC):
                wt = wo_pool.tile([128, E], BF, name=f"wo{h}", tag=f"wo{h}")
                nc.sync.dma_start(wt[:], wo[128 * h : 128 * (h + 1), :])
                wos.append(wt)
            nec = E // 512
            for i in range(NB):
                psums = [
                    ps_out.tile([128, 512], FP, name="pso", tag="pso")
                    for _ in range(nec)
                ]
                for h in range(H_LOC):
                    ah_blk = attO[h][:, 128 * i : 128 * (i + 1)]
                    for c in range(nec):
                        nc.tensor.matmul(
                            psums[c][:],
                            ah_blk,
                            wos[h][:, 512 * c : 512 * (c + 1)],
                            start=(h == 0),
                            stop=(h == H_LOC - 1),
                        )
                for c in range(nec):
                    ot = ostg.tile([128, 512], BF, name="ostg", tag="ostg")
                    if c % 2 == 0:
                        nc.vector.tensor_copy(ot[:], psums[c][:])
                    else:
                        nc.scalar.activation(ot[:], psums[c][:], AF.Copy)
                    dq = nc.sync if c % 2 == 0 else nc.scalar
                    dq.dma_start(
                        out[128 * i : 128 * (i + 1), 512 * c : 512 * (c + 1)],
                        ot[:],
                    )


_NC_CACHE = None


def _get_nc():
    global _NC_CACHE
    if _NC_CACHE is None:
        nc = bacc.Bacc(
            "TRN2",
            target_bir_lowering=False,
            debug=False,
            num_devices=1,
            enable_asserts=False,
        )
        _emit(nc)
        nc.compile()
        _NC_CACHE = nc
    return _NC_CACHE


def make_in_maps(inX, W_qkv, b_qkv, W_out):
    bf = ml_dtypes.bfloat16
    xTs = [np.ascontiguousarray(inX[b].T.astype(bf)) for b in range(B)]
    in_maps = []
    for c in range(NCORES):
        b = c // HG
        hg = c % HG
        sl = slice(FLOC * hg, FLOC * (hg + 1))
        in_maps.append(
            {
                "xT": xTs[b],
                "wq": np.ascontiguousarray(
                    (W_qkv[:, 0:E][:, sl] * SCALE).astype(bf)
                ),
                "wk": np.ascontiguousarray(W_qkv[:, E : 2 * E][:, sl].astype(bf)),
                "wv": np.ascontiguousarray(W_qkv[:, 2 * E : 3 * E][:, sl].astype(bf)),
                "bqs": np.ascontiguousarray(
                    (b_qkv[0:E][sl] * SCALE).reshape(FLOC, 1).astype(np.float32)
                ),
                "bk": np.ascontiguousarray(
                    b_qkv[E : 2 * E][sl].reshape(FLOC, 1).astype(np.float32)
                ),
                "wo": np.ascontiguousarray(W_out[sl, :].astype(bf)),
            }
        )
    return in_maps


def kernel(inX, W_qkv, b_qkv, W_out, b_out):
    global LAST_EXEC_NS, LAST_RESULTS
    inX = np.asarray(inX, dtype=np.float32)
    W_qkv = np.asarray(W_qkv, dtype=np.float32)
    b_qkv = np.asarray(b_qkv, dtype=np.float32)
    W_out = np.asarray(W_out, dtype=np.float32)
    b_out = np.asarray(b_out, dtype=np.float32)

    nc = _get_nc()
    in_maps = make_in_maps(inX, W_qkv, b_qkv, W_out)

    kwargs = {}
    if PROFILE:
        kwargs = {"trace": True, "trace_cores": [0]}
    res = bass_utils.run_bass_kernel_spmd(
        nc, in_maps, core_ids=list(range(NCORES)), **kwargs
    )
    LAST_EXEC_NS = res.exec_time_ns
    LAST_RESULTS = res

    bias_full = (b_out + b_qkv[2 * E : 3 * E] @ W_out).astype(np.float32)
    out = np.empty((B, S, E), dtype=np.float32)
    for b in range(B):
        acc = res.results[HG * b + 0]["out"].astype(np.float32)
        for hg in range(1, HG):
            acc += res.results[HG * b + hg]["out"].astype(np.float32)
        out[b] = acc + bias_full
    return out
